# revision 41
# baseline (speedup 1.0000x reference)
"""Trainium2 Bass kernel for nn_CustomGNN (edge-MLP message passing + segment mean).

Strategy (8 NeuronCores, SPMD):
  - Host sorts edges by destination (obj) and shards them by obj node range
    (12500 nodes/core), so each core owns a disjoint slice of the output and
    no cross-core reduction is needed.
  - Edges are packed into 128-edge subtiles, node-aligned (no node's edges
    straddle a subtile), so per-edge messages stream out in sorted-by-obj
    order and the host can finish with a cheap segment mean (reduceat).
  - Per core-half, the host rank-compresses the referenced node set (<=65534
    unique) into a private permuted bf16 table of 65536 rows, enabling the
    GpSimd dma_gather custom instruction (int16 indices) to reach the whole
    table in ONE call via uint16 wraparound addressing (base +32768,
    idx16 = j ^ 0x8000).  One 2048-row gather per (group, role) -- larger
    gathers hit SWDGE descriptor-ring backpressure -- rotating over 4 queues.
  - The first HEADPRE groups are pre-gathered by the host and arrive via
    plain DMA, hiding the ~44us cold GpSimd library load at kernel start.
  - Device: gather rows [edge, feat] -> PE transpose -> 3-layer MLP with fp8
    DoubleRow L1 AND L2 (fp32 PSUM accumulation) -> L3 emitted transposed
    ([D, edge], one 512-wide matmul per tile) -> per-edge messages (bf16,
    two tiles batched per DMA) to DRAM.  (dma_gather transpose=True was
    tried and is fast, but the shared per-core XBAR corrupts data
    nondeterministically with several transposed gathers in flight;
    indirect_dma_start NaNs/crawls at 2048 rows -- use neither.)
  - Host divides by counts, adds b3, and scatters node means to output rows.
"""
import os
import sys
import time

sys.path.insert(0, "/opt/trn_rl_repo")

import numpy as np
import ml_dtypes

bf16 = ml_dtypes.bfloat16

# problem sizes (hardcoded per contract)
N, E, D = 100000, 300000, 128
H1, H2 = 512, 64
NC = 8                  # cores
NPC = N // NC           # nodes per core
ST = 304                # subtiles per core (128 edges each)
NG = 19                 # gather groups (16 subtiles = 2048 edges each)
TPG = 4                 # MLP tiles (512 edges) per group
GIDX = 3 * 2048         # gathered rows per group (3 roles x 2048 edges)
ICOLS = GIDX // 16      # wrapped idx columns per group (384)
EPC = ST * 128          # padded edges per core
TAB = 65536             # rows per half-table
DUMMY_J = 32768         # table row reserved as the zero row
HALF_GROUPS = 10        # groups 0..9 -> half 0, 10..18 -> half 1
HEADPRE = 5             # leading groups pre-gathered by the host (plain DMA
                        # while the GpSimd library loads, ~44us)

_COMPILED = None        # (nc, meta) cache across kernel() calls
last_exec_time_ns = None


def _wrap_idx_group(idx16):
    """[GIDX] int16 -> [128, ICOLS] wrapped for dma_gather.

    wrapped[p, col] = idx[16*col + (p % 16)], replicated across the 8
    16-partition groups.
    """
    w16 = idx16.reshape(ICOLS, 16).T               # [16, ICOLS]
    return np.tile(w16, (8, 1))                    # [128, ICOLS]


def _prep_core(o, p, s):
    """Pack one core's (sorted-by-obj) edges. Returns per-core arrays."""
    ne = len(o)
    nodes, starts, counts = np.unique(o, return_index=True, return_counts=True)
    assert counts.max() <= 127, f"node degree {counts.max()} exceeds subtile capacity"

    # greedy node-aligned packing; group-end subtiles (every 16th) keep one
    # dummy slot so every gather's trailing index is the (non-negative int16)
    # dummy row
    sub_of_node = np.empty(len(nodes), np.int32)
    pos_of_node = np.empty(len(nodes), np.int32)
    st, fill = 0, 0
    for i in range(len(nodes)):
        c = counts[i]
        cap = 127 if (st % 16) == 15 else 128
        if fill + c > cap:
            st += 1
            fill = 0
        sub_of_node[i] = st
        pos_of_node[i] = fill
        fill += c
    assert st < ST, f"needs {st + 1} subtiles > {ST}"

    edge_sub = np.repeat(sub_of_node, counts)
    edge_pos = np.repeat(pos_of_node, counts) + (np.arange(ne) - np.repeat(starts, counts))

    eidx = np.full((ST, 128), -1, np.int64)
    eidx[edge_sub, edge_pos] = np.arange(ne)
    mask = eidx >= 0

    gnode = np.full((3, ST, 128), -1, np.int64)
    for r, arr in enumerate((o, p, s)):
        g = arr[np.clip(eidx, 0, None)]
        g[~mask] = -1
        gnode[r] = g
    return gnode, mask, nodes, starts, counts


def _prep_tables(gnode, x_bf):
    """Build per-half permuted tables + wrapped idx arrays for one core."""
    tables = []
    idxw = np.zeros((128, NG * ICOLS), np.int16)
    for h in range(2):
        lo = 0 if h == 0 else HALF_GROUPS * 16
        hi = HALF_GROUPS * 16 if h == 0 else ST
        ids = gnode[:, lo:hi, :]
        uniq = np.unique(ids[ids >= 0])
        assert len(uniq) <= 65534, f"half {h}: {len(uniq)} unique nodes"
        T = np.zeros((TAB, D), bf16)
        jmap = np.arange(len(uniq), dtype=np.int64)
        jmap = jmap + (jmap >= DUMMY_J)
        T[jmap] = x_bf[uniq]
        tables.append(T)
        glo = 0 if h == 0 else HALF_GROUPS
        ghi = HALF_GROUPS if h == 0 else NG
        for g in range(glo, ghi):
            ids_g = gnode[:, g * 16:(g + 1) * 16, :].reshape(GIDX)
            j = np.full(GIDX, DUMMY_J, np.int64)
            real = ids_g >= 0
            rank = np.searchsorted(uniq, ids_g[real])
            j[real] = rank + (rank >= DUMMY_J)
            i16 = (j.astype(np.uint16) ^ 0x8000).view(np.int16)
            idxw[:, g * ICOLS:(g + 1) * ICOLS] = _wrap_idx_group(i16)
    return tables[0], tables[1], idxw


def _build_program():
    import concourse.bass as bass
    import concourse.tile as tile
    import concourse.bacc as bacc
    import concourse.mybir as mybir
    from concourse.library_config import mlp as mlp_lib

    f32 = mybir.dt.float32
    b16 = mybir.dt.bfloat16
    fp8 = mybir.dt.float8e4
    Relu = mybir.ActivationFunctionType.Relu
    Copy = mybir.ActivationFunctionType.Copy
    DR = mybir.MatmulPerfMode.DoubleRow

    nc = bacc.Bacc("TRN2", target_bir_lowering=False, debug=False,
                   num_devices=NC, num_swdge_queues=4)
    t0 = nc.dram_tensor("t0", [TAB, D], b16, kind="ExternalInput").ap()
    t1 = nc.dram_tensor("t1", [TAB, D], b16, kind="ExternalInput").ap()
    idxw = nc.dram_tensor("idxw", [128, NG * ICOLS], mybir.dt.int16, kind="ExternalInput").ap()
    head = nc.dram_tensor("head", [HEADPRE, 3, 128, 16, 128], b16, kind="ExternalInput").ap()
    w1dr = nc.dram_tensor("w1dr", [128, 2, H1], fp8, kind="ExternalInput").ap()
    w1c = nc.dram_tensor("w1c", [128, H1], fp8, kind="ExternalInput").ap()
    w2dr = nc.dram_tensor("w2dr", [128, 2, 2, H2], fp8, kind="ExternalInput").ap()
    w3t = nc.dram_tensor("w3t", [128, D], b16, kind="ExternalInput").ap()
    b1s = nc.dram_tensor("b1s", [128, 4], f32, kind="ExternalInput").ap()
    b2s = nc.dram_tensor("b2s", [128, 1], f32, kind="ExternalInput").ap()
    ident = nc.dram_tensor("ident", [128, 128], b16, kind="ExternalInput").ap()
    sstream = nc.dram_tensor("sstream", [NG * TPG // 2, 128, 2, 512], b16, kind="ExternalOutput").ap()

    with tile.TileContext(nc) as tc:
        with tc.tile_pool(name="const", bufs=1) as cp, \
             tc.tile_pool(name="gb", bufs=4) as gb, \
             tc.tile_pool(name="ft", bufs=18) as ftp, \
             tc.tile_pool(name="h1", bufs=8) as h1p, \
             tc.tile_pool(name="h2", bufs=4) as h2p, \
             tc.tile_pool(name="msg", bufs=8) as msgp, \
             tc.tile_pool(name="pf", bufs=2, space="PSUM") as pf, \
             tc.tile_pool(name="ph1", bufs=2, space="PSUM") as ph1, \
             tc.tile_pool(name="p2p", bufs=1, space="PSUM") as p2p, \
             tc.tile_pool(name="pm", bufs=2, space="PSUM") as pm:

            nc.gpsimd.load_library(mlp_lib)

            # ident first: the very first transposes need it, and anything
            # queued ahead of it on the ring delays the whole pipeline
            id_sb = cp.tile([128, 128], b16)
            nc.sync.dma_start(id_sb[:], ident[:])

            HEADG = 2
            HEADC = HEADG * ICOLS
            idx_a = cp.tile([128, HEADC], mybir.dt.int16)
            idx_b = cp.tile([128, (NG - HEADG) * ICOLS], mybir.dt.int16)

            RCOLS = 2048 // 16

            def idx_slice(g, r):
                base = (g - (0 if g < HEADG else HEADG)) * ICOLS + r * RCOLS
                src = idx_a if g < HEADG else idx_b
                return src[:, base:base + RCOLS]

            w1dr_sb = cp.tile([128, 2, H1], fp8)
            nc.sync.dma_start(w1dr_sb[:], w1dr[:])
            w1c_sb = cp.tile([128, H1], fp8)
            nc.sync.dma_start(w1c_sb[:], w1c[:])
            w2_sb = cp.tile([128, 2, 2, H2], fp8)
            nc.sync.dma_start(w2_sb[:], w2dr[:])
            w3_sb = cp.tile([128, D], b16)
            nc.sync.dma_start(w3_sb[:], w3t[:])
            b1_sb = cp.tile([128, 4], f32)
            nc.sync.dma_start(b1_sb[:], b1s[:])
            b2_sb = cp.tile([128, 1], f32)
            nc.sync.dma_start(b2_sb[:], b2s[:])

            qn = 0
            for g in range(NG):
                tab = t0 if g < HALF_GROUPS else t1
                # one gather per (group, role): 2048 rows each.  Leading
                # groups come host-pre-gathered via plain DMA so the PE can
                # start while the GpSimd library loads.
                gts = []
                gq0 = None
                if g == 0:
                    # group 0 arrives as 12 independent 128KB quad tiles so
                    # the first transpose only waits for one small DMA
                    gq0 = [[gb.tile([128, 4, 128], b16, tag=f"q{r}{t}",
                                    name=f"gq0_{r}_{t}")
                            for t in range(TPG)] for r in range(3)]
                    for t in range(TPG):
                        for r in range(3):
                            nc.sync.dma_start(
                                gq0[r][t][:], head[g, r, :, t * 4:(t + 1) * 4, :])
                else:
                    for r in range(3):
                        gtr = gb.tile([128, 16, 128], b16, tag=f"g{r}")
                        if g < HEADPRE:
                            nc.sync.dma_start(gtr[:], head[g, r])
                        else:
                            nc.gpsimd.dma_gather(
                                gtr[:], tab[DUMMY_J:, :], idx_slice(g, r),
                                2048, 2048, 128, transpose=False,
                                single_packet=False, queue_num=qn)
                            qn = (qn + 1) % 4
                        gts.append(gtr)
                # featsT[f, e] per tile via PE transposes; cast to fp8 in the
                # PSUM->SBUF copy. Roles 0,1 interleave into one [128,2,512]
                # tile (DoubleRow rhs layout), role 2 gets its own tile.
                ftds = []
                ft2s = []
                for t in range(TPG):
                    ftd = ftp.tile([128, 2, 512], fp8, tag="ftd")
                    ft2 = ftp.tile([128, 512], fp8, tag="ft2")
                    for r in range(3):
                        pft = pf.tile([128, 512], b16, tag="pf")
                        srcq = gq0[r][t] if g == 0 else None
                        for u in range(4):
                            nc.tensor.transpose(
                                pft[:, u * 128:(u + 1) * 128],
                                srcq[:, u, :] if g == 0 else gts[r][:, t * 4 + u, :],
                                id_sb[:])
                        if r < 2:
                            nc.scalar.activation(ftd[:, r, :], pft[:], Copy)
                        else:
                            nc.vector.tensor_copy(ft2[:], pft[:])
                    ftds.append(ftd)
                    ft2s.append(ft2)
                if g == 1:
                    # idx rides behind the first head tiles; first real
                    # gather (group HEADPRE) is ~40us out
                    nc.sync.dma_start(idx_a[:], idxw[:, :HEADC])
                    nc.sync.dma_start(idx_b[:], idxw[:, HEADC:])

                # L1 weight-stationary across the group's 4 tiles
                # (DoubleRow fp8: roles 0,1 as the 2 k-tiles; role 2 plain fp8)
                h1s = [h1p.tile([128, 4, 512], fp8, tag="h1", name=f"h1_{g}_{i}") for i in range(TPG)]
                for m in range(4):
                    for half in range(2):
                        p1s = [ph1.tile([128, 512], f32, tag="ph1",
                                        name=f"p1_{g}_{m}_{half}_{i}") for i in range(2)]
                        ts = (half * 2, half * 2 + 1)
                        for i, t in enumerate(ts):
                            nc.tensor.matmul(
                                p1s[i][:], lhsT=w1dr_sb[:, :, m * 128:(m + 1) * 128],
                                rhs=ftds[t][:], start=True, stop=False,
                                perf_mode=DR)
                        for i, t in enumerate(ts):
                            nc.tensor.matmul(
                                p1s[i][:], lhsT=w1c_sb[:, m * 128:(m + 1) * 128],
                                rhs=ft2s[t][:], start=False, stop=True)
                        for i, t in enumerate(ts):
                            if t >= 2:
                                nc.vector.tensor_scalar(
                                    out=h1s[t][:, m, :], in0=p1s[i][:],
                                    scalar1=b1_sb[:, m:m + 1], scalar2=0.0,
                                    op0=mybir.AluOpType.add, op1=mybir.AluOpType.max)
                            else:
                                nc.scalar.activation(h1s[t][:, m, :], p1s[i][:], Relu,
                                                     bias=b1_sb[:, m:m + 1], scale=1.0)
                for tp in range(0, TPG, 2):
                    # L2 for tile pair (tp, tp+1): fp8 DoubleRow over the 4
                    # k-tiles (2 DR matmuls each); DR dst must start at
                    # partition 0, so each tile gets its own [64, 512] PSUM
                    p2s = [p2p.tile([H2, 512], f32, tag=f"p2{i}",
                                    name=f"p2_{g}_{tp}_{i}") for i in range(2)]
                    for j in range(2):
                        for i in range(2):
                            nc.tensor.matmul(p2s[i][:], lhsT=w2_sb[:, j, :, :],
                                             rhs=h1s[tp + i][:, 2 * j:2 * j + 2, :],
                                             start=(j == 0), stop=(j == 1),
                                             perf_mode=DR)
                    h2 = h2p.tile([128, 512], b16, tag="h2")
                    for i in range(2):
                        nc.scalar.activation(h2[i * H2:(i + 1) * H2, :], p2s[i][:],
                                             Relu, bias=b2_sb[0:H2, 0:1],
                                             scale=1.0 / 1024.0)
                    # L3 transposed: out[D, e] = W3T-slice.T @ h2-slice, one
                    # 512-wide matmul per tile; the two tiles use disjoint
                    # k-row-groups (0-63 / 64-127) and run concurrently
                    pmts = [pm.tile([128, 512], f32, tag="pm",
                                    name=f"pm_{g}_{tp}_{i}") for i in range(2)]
                    for ti in range(2):
                        plo, phi = ti * H2, (ti + 1) * H2
                        nc.tensor.matmul(pmts[ti][:], lhsT=w3_sb[plo:phi, :],
                                         rhs=h2[plo:phi, :], start=True, stop=True)
                    msg = msgp.tile([128, 2, 512], b16, tag="msg")
                    for ti in range(2):
                        nc.vector.tensor_copy(msg[:, ti], pmts[ti][:])
                    nc.sync.dma_start(sstream[g * 2 + tp // 2], msg[:])

    nc.compile()
    return nc


def kernel(x, edge_index, W1, b1, W2, b2, W3, b3, **_):
    global _COMPILED, last_exec_time_ns
    from concourse.bass_utils import run_bass_kernel_spmd

    x = np.ascontiguousarray(np.asarray(x, dtype=np.float32))
    ei = np.asarray(edge_index)
    ei = ei.astype(np.int64)
    W1 = np.asarray(W1, np.float32); b1 = np.asarray(b1, np.float32)
    W2 = np.asarray(W2, np.float32); b2 = np.asarray(b2, np.float32)
    W3 = np.asarray(W3, np.float32); b3 = np.asarray(b3, np.float32)

    obj, pred, sub = ei[:, 0], ei[:, 1], ei[:, 2]
    order = np.argsort(obj, kind="stable")
    obj_s, pred_s, sub_s = obj[order], pred[order], sub[order]
    bounds = np.searchsorted(obj_s, np.arange(NC + 1) * NPC)
    x_bf = x.astype(bf16)
    xtab = np.vstack([x_bf, np.zeros((1, D), bf16)])   # row N = zeros

    # shared constant tensors.  L1 runs in fp8 e4m3 with W1 scaled by 64
    # (lifts the tiny uniform(+-1/sqrt(384)) weights out of the subnormal
    # range); h1 comes out scaled by 64 and is stored as fp8 for the L2
    # DoubleRow matmuls with W2 scaled by 16; the combined 1024x factor is
    # unwound in the h2 activation scale.
    fp8e4 = ml_dtypes.float8_e4m3fn
    W1T = np.ascontiguousarray(W1.T) * 64.0                # [384, 512] scaled
    W1r = W1T.reshape(3, 128, H1)
    w1dr = np.ascontiguousarray(W1r[0:2].transpose(1, 0, 2)).astype(fp8e4)
    w1c = np.ascontiguousarray(W1r[2]).astype(fp8e4)       # [128, 512]
    W2T = np.ascontiguousarray(W2.T) * 16.0                # [512, 64] scaled
    W2r = W2T.reshape(2, 2, 128, H2)                       # [j, ko, p, h]
    w2dr = np.ascontiguousarray(W2r.transpose(2, 0, 1, 3)).astype(fp8e4)
    w3t = np.ascontiguousarray(np.concatenate([W3.T, W3.T], axis=0)).astype(bf16)  # [128, 128]
    b1s = np.ascontiguousarray(b1.reshape(4, 128).T).astype(np.float32) * 64.0
    b2s = np.concatenate([b2, b2]).reshape(128, 1).astype(np.float32)
    ident = np.eye(128, dtype=np.float32).astype(bf16)

    in_maps = []
    metas = []
    for c in range(NC):
        lo, hi = bounds[c], bounds[c + 1]
        gnode, mask, nodes, starts, counts = _prep_core(
            obj_s[lo:hi], pred_s[lo:hi], sub_s[lo:hi])
        T0, T1, idxw_c = _prep_tables(gnode, x_bf)
        hg = gnode[:, :HEADPRE * 16, :].reshape(3, HEADPRE, 16, 128)
        head_c = np.ascontiguousarray(
            xtab[np.where(hg < 0, N, hg)].transpose(1, 0, 3, 2, 4))
        in_maps.append({
            "t0": T0, "t1": T1, "idxw": idxw_c, "head": head_c,
            "w1dr": w1dr, "w1c": w1c, "w2dr": w2dr, "w3t": w3t,
            "b1s": b1s, "b2s": b2s, "ident": ident,
        })
        metas.append((mask, nodes, starts, counts))

    if _COMPILED is None:
        _COMPILED = _build_program()
    nc = _COMPILED

    trace = os.environ.get("GNN_TRACE", "0") == "1"
    if trace:
        try:
            import antenv.axon_hooks  # noqa: F401  (absent on bare images)
        except ImportError:
            trace = False
    tdir = os.environ.get("GNN_TRACE_DIR") if trace else None
    if tdir:
        os.makedirs(tdir, exist_ok=True)
    res = run_bass_kernel_spmd(nc, in_maps, list(range(NC)), trace=trace,
                               tmpdir=tdir)
    last_exec_time_ns = res.exec_time_ns
    if trace and res.exec_time_ns:
        print(f"HW exec time: {res.exec_time_ns} ns")

    # host finalize: per-edge messages -> segment mean + b3 -> node rows
    out = x.copy()
    for c in range(NC):
        mask, nodes, starts, counts = metas[c]
        stream = res.results[c]["sstream"]                 # [38, 128(D), 2, 512(e)] bf16
        stream = np.ascontiguousarray(stream.transpose(0, 2, 3, 1)).reshape(EPC, D)
        msgs = stream[mask.reshape(-1)].astype(np.float32)  # [ne, D] in obj order
        sums = np.add.reduceat(msgs, starts, axis=0)
        out[nodes] = sums / counts[:, None] + b3
    return out
